# revision 1
# baseline (speedup 1.0000x reference)
"""Builder for the DecomposableAttention Trainium2 kernel.

Layouts (per core, NB batches):
  x1t/x2t : [NB, E=512, L=256]  (host-transposed sentences; f32r)   -> f-MLP rhs
  x1n/x2n : [NB, L=256, E=512]  (natural; f32r)                     -> att lhsT
  weights : transposed on host so W^T k-tiles DMA straight in.

All matmuls run in float32r (20-bit float stored as fp32 with low 12 bits
zero).  PE accumulates fp32 into PSUM.  Host pre-rounds every DRAM input;
every on-chip producer feeding a matmul writes a float32r-typed tile.

g = batches per MLP weight pass: the f/g MLP moving operands are g*L wide,
amortizing the fused f32r weight loads over g batches.  e1 / softmax / att
stay per-batch (their operands differ per batch).
"""

import sys

for p in ("/opt/trn_rl_repo", "/root/.axon_site/_ro/trn_rl_repo"):
    if p not in sys.path:
        sys.path.insert(0, p)

import numpy as np
import concourse.bass as bass
import concourse.mybir as mybir
from concourse import bacc
from concourse.tile import TileContext

dt = mybir.dt
AF = mybir.ActivationFunctionType
AX = mybir.AxisListType

B, L, E, H, OUT = 128, 256, 512, 1024, 3
NCORES = 8
NB = B // NCORES          # batches per core
KE = E // 128             # 4  k-tiles over E
KH = H // 128             # 8  k-tiles over H
KL = L // 128             # 2  k-tiles over L
MMDT = dt.float32r


def round_f32r(x: np.ndarray) -> np.ndarray:
    """Round fp32 array to the FP32R grid (11-bit mantissa, RNE)."""
    xi = np.ascontiguousarray(x, dtype=np.float32).view(np.uint32).astype(np.uint64)
    lsb = (xi >> 12) & 1
    r = (xi + 0x7FF + lsb) & 0xFFFFF000
    return r.astype(np.uint32).view(np.float32)


def build_nc(nb=NB, g=1, debug_taps=(), stage=6):
    assert nb % g == 0
    nc = bacc.Bacc("TRN2", target_bir_lowering=False)
    GL = g * L

    def param(name, shape, dtype=MMDT, out=False):
        return nc.declare_dram_parameter(name, list(shape), dtype, isOutput=out)

    x1t = param("x1t", [nb, E, L])
    x2t = param("x2t", [nb, E, L])
    x1n = param("x1n", [nb, L, E])
    x2n = param("x2n", [nb, L, E])
    fw1t = param("fw1t", [E, H])
    fw2t = param("fw2t", [H, H])
    gw1t = param("gw1t", [H, H])     # rows 0:512 att part, 512:1024 x part
    gw2t = param("gw2t", [H, H])
    hw1t = param("hw1t", [2 * H, H])
    hw2t = param("hw2t", [H, H])
    finwt = param("finwt", [H, 4])
    fb1 = param("fb1", [128, KH], dt.float32)
    fb2 = param("fb2", [128, KH], dt.float32)
    gb1 = param("gb1", [128, KH], dt.float32)
    gb2 = param("gb2", [128, KH], dt.float32)
    hb1 = param("hb1", [128, KH], dt.float32)
    hb2 = param("hb2", [128, KH], dt.float32)
    finb = param("finb", [4, 1], dt.float32)
    ident_in = param("ident_in", [128, 128])
    out_d = param("out", [4, nb], dt.float32, out=True)
    tap_shapes = {"f1t": [128, KH, L], "e1": [128, KL, L], "w1a": [128, KL, L],
                  "att1": [128, KE, L]}
    taps = {t: param(f"tap_{t}", tap_shapes[t], dt.float32, out=True)
            for t in debug_taps}

    with TileContext(nc) as tc, \
         tc.tile_pool(name="wpool", bufs=1) as wpool:
        ident = wpool.tile([128, 128], MMDT)
        nc.sync.dma_start(out=ident, in_=ident_in[:])
        s_allT = wpool.tile([128, 2 * KH, nb], dt.float32)  # aggregate input

        def mm_group(psum, lhs_fn, rhs_fn, nk):
            for k in range(nk):
                nc.tensor.matmul(psum, lhs_fn(k), rhs_fn(k),
                                 start=(k == 0), stop=(k == nk - 1))

        def mlp2(pools, rhs1_fn, nk1, w1_sb, b1_sb, w2_sb, b2_sb, out_sb,
                 width, accum_cols=None):
            """Two-layer ReLU MLP on transposed activations ([128, k, width]).
            accum_cols(gi, m) -> [128, 1] AP: layer-2 relu row-sums per batch."""
            ps, hidp, scr = pools
            hid = hidp.tile([128, KH, width], MMDT, name="mlp_hid", tag="mlp_hid")
            for m in range(KH):
                psum = ps.tile([128, width], dt.float32, name="mlp_ps", tag="mlp_ps")
                mm_group(psum, lambda k, m=m: w1_sb[:, k, m * 128:(m + 1) * 128],
                         rhs1_fn, nk1)
                nc.scalar.activation(out=hid[:, m], in_=psum, func=AF.Relu,
                                     bias=b1_sb[:, m:m + 1], scale=1.0)
            for m in range(KH):
                psum = ps.tile([128, width], dt.float32, name="mlp_ps", tag="mlp_ps")
                mm_group(psum, lambda k, m=m: w2_sb[:, k, m * 128:(m + 1) * 128],
                         lambda k: hid[:, k], KH)
                if accum_cols is None:
                    nc.scalar.activation(out=out_sb[:, m], in_=psum, func=AF.Relu,
                                         bias=b2_sb[:, m:m + 1], scale=1.0)
                else:
                    for gi in range(width // L):
                        o = scr.tile([128, L], dt.float32, name="g_scr",
                                     tag="g_scr", bufs=3)
                        nc.scalar.activation(
                            out=o, in_=psum[:, gi * L:(gi + 1) * L], func=AF.Relu,
                            bias=b2_sb[:, m:m + 1], scale=1.0,
                            accum_out=accum_cols(gi, m))

        def transpose256(in_sb2, out_sb2, ps_tr):
            """[128, KL, 256] -> full 256x256 transpose via 4 PE transposes.
            Returns the psum tiles; copies to out_sb2 when given."""
            outs = []
            for a in range(KL):
                ptr = ps_tr.tile([128, L], MMDT, name="tr_ps", tag="tr_ps")
                for bq in range(KL):
                    nc.tensor.transpose(ptr[:, bq * 128:(bq + 1) * 128],
                                        in_sb2[:, bq, a * 128:(a + 1) * 128], ident)
                outs.append(ptr)
                if out_sb2 is not None:
                    nc.vector.tensor_copy(out_sb2[:, a], ptr)
            return outs

        with tc.tile_pool(name="fgw", bufs=1) as fgw, \
             tc.tile_pool(name="xp", bufs=2) as xp, \
             tc.tile_pool(name="xnp", bufs=2) as xnp, \
             tc.tile_pool(name="fp", bufs=1) as fp, \
             tc.tile_pool(name="hidp", bufs=2) as hidp, \
             tc.tile_pool(name="smp", bufs=1) as smp, \
             tc.tile_pool(name="attp", bufs=1) as attp, \
             tc.tile_pool(name="scrp", bufs=3) as scrp, \
             tc.tile_pool(name="ps", bufs=4, space="PSUM") as ps, \
             tc.tile_pool(name="ps_e", bufs=2, space="PSUM") as ps_e, \
             tc.tile_pool(name="ps_tr", bufs=2, space="PSUM") as ps_tr:
            pools = (ps, hidp, scrp)
            fw1_sb = fgw.tile([128, KE, H], MMDT)
            fw2_sb = fgw.tile([128, KH, H], MMDT)
            gw1_sb = fgw.tile([128, KH, H], MMDT)
            gw2_sb = fgw.tile([128, KH, H], MMDT)
            nc.sync.dma_start(out=fw1_sb, in_=fw1t[:].rearrange("(k p) m -> p k m", p=128))
            nc.sync.dma_start(out=fw2_sb, in_=fw2t[:].rearrange("(k p) m -> p k m", p=128))
            nc.sync.dma_start(out=gw1_sb, in_=gw1t[:].rearrange("(k p) m -> p k m", p=128))
            nc.sync.dma_start(out=gw2_sb, in_=gw2t[:].rearrange("(k p) m -> p k m", p=128))
            fb1_sb = fgw.tile([128, KH], dt.float32)
            fb2_sb = fgw.tile([128, KH], dt.float32)
            gb1_sb = fgw.tile([128, KH], dt.float32)
            gb2_sb = fgw.tile([128, KH], dt.float32)
            nc.sync.dma_start(out=fb1_sb, in_=fb1[:])
            nc.sync.dma_start(out=fb2_sb, in_=fb2[:])
            nc.sync.dma_start(out=gb1_sb, in_=gb1[:])
            nc.sync.dma_start(out=gb2_sb, in_=gb2[:])
            for bg in range(nb // g):
                bs = [bg * g + i for i in range(g)]
                # ---- load inputs ----
                x1t_sb = xp.tile([128, KE, GL], MMDT, name="x1t_sb", tag="x1t")
                x2t_sb = xp.tile([128, KE, GL], MMDT, name="x2t_sb", tag="x2t")
                x1n_sb = xnp.tile([128, KL * g, E], MMDT, name="x1n_sb", tag="x1n")
                x2n_sb = xnp.tile([128, KL * g, E], MMDT, name="x2n_sb", tag="x2n")
                for gi, b in enumerate(bs):
                    nc.sync.dma_start(
                        out=x1t_sb[:, :, gi * L:(gi + 1) * L],
                        in_=x1t[b].rearrange("(k p) l -> p k l", p=128))
                    nc.sync.dma_start(
                        out=x2t_sb[:, :, gi * L:(gi + 1) * L],
                        in_=x2t[b].rearrange("(k p) l -> p k l", p=128))
                    nc.sync.dma_start(
                        out=x1n_sb[:, gi * KL:(gi + 1) * KL, :],
                        in_=x1n[b].rearrange("(k p) e -> p k e", p=128))
                    nc.sync.dma_start(
                        out=x2n_sb[:, gi * KL:(gi + 1) * KL, :],
                        in_=x2n[b].rearrange("(k p) e -> p k e", p=128))

                # ---- attend MLP f (both sentences, g batches wide) ----
                f1t = fp.tile([128, KH, GL], MMDT, name="f1t", tag="f1t")
                f2t = fp.tile([128, KH, GL], MMDT, name="f2t", tag="f2t")
                mlp2(pools, lambda k: x1t_sb[:, k], KE, fw1_sb, fb1_sb,
                     fw2_sb, fb2_sb, f1t, GL)
                mlp2(pools, lambda k: x2t_sb[:, k], KE, fw1_sb, fb1_sb,
                     fw2_sb, fb2_sb, f2t, GL)
                if "f1t" in taps and bg == 0:
                    nc.sync.dma_start(out=taps["f1t"][:],
                                      in_=f1t.bitcast(dt.float32)[:, :, 0:L])

                # per-batch attention; att tiles span the group (GL wide)
                att1 = attp.tile([128, KE, GL], MMDT, name="att1", tag="att1")
                att2 = attp.tile([128, KE, GL], MMDT, name="att2", tag="att2")
                for gi, b in enumerate(bs):
                    if stage < 2:
                        break
                    gl = slice(gi * L, (gi + 1) * L)
                    # ---- e1 = f1 @ f2^T ----
                    e1sb = smp.tile([128, KL, L], MMDT, name="e1sb", tag="e1sb")
                    w1a = smp.tile([128, KL, L], MMDT, name="w1a", tag="w1a")
                    for im in range(KL):
                        pe1 = ps_e.tile([128, L], dt.float32, name="pe1", tag="pe1")
                        mm_group(pe1,
                                 lambda k, im=im: f1t[:, k, gi * L + im * 128:
                                                      gi * L + (im + 1) * 128],
                                 lambda k: f2t[:, k, gl], KH)
                        # row softmax (over j = free dim)
                        nmax = scrp.tile([128, 1], dt.float32, name="nmax", tag="sm1")
                        nc.vector.reduce_max(out=nmax, in_=pe1, axis=AX.X, negate=True)
                        ex = scrp.tile([128, L], dt.float32, name="ex", tag="smE")
                        rs = scrp.tile([128, 1], dt.float32, name="rs", tag="sm2")
                        nc.scalar.activation(out=ex, in_=pe1, func=AF.Exp,
                                             bias=nmax, scale=1.0, accum_out=rs)
                        rr = scrp.tile([128, 1], dt.float32, name="rr", tag="sm3")
                        nc.vector.reciprocal(out=rr, in_=rs)
                        nc.vector.tensor_scalar_mul(w1a[:, im], ex, rr)
                        nc.vector.tensor_copy(e1sb[:, im], pe1)
                    if "e1" in taps and b == 0:
                        nc.sync.dma_start(out=taps["e1"][:],
                                          in_=e1sb.bitcast(dt.float32))
                    if "w1a" in taps and b == 0:
                        nc.sync.dma_start(out=taps["w1a"][:],
                                          in_=w1a.bitcast(dt.float32))
                    if stage < 3:
                        continue
                    # ---- e1T -> col softmax -> w2a [j, i] ----
                    w2a = smp.tile([128, KL, L], MMDT, name="w2a", tag="w2a")
                    e1t_ps = transpose256(e1sb, None, ps_tr)
                    for jm in range(KL):
                        pet = e1t_ps[jm]
                        nmax2 = scrp.tile([128, 1], dt.float32, name="nmax2", tag="sm1")
                        nc.vector.reduce_max(out=nmax2, in_=pet, axis=AX.X, negate=True)
                        ex2 = scrp.tile([128, L], dt.float32, name="ex2", tag="smE")
                        cs = scrp.tile([128, 1], dt.float32, name="cs", tag="sm2")
                        nc.scalar.activation(out=ex2, in_=pet, func=AF.Exp,
                                             bias=nmax2, scale=1.0, accum_out=cs)
                        rc = scrp.tile([128, 1], dt.float32, name="rc", tag="sm3")
                        nc.vector.reciprocal(out=rc, in_=cs)
                        nc.vector.tensor_scalar_mul(w2a[:, jm], ex2, rc)
                    # ---- transpose weights for att matmuls ----
                    w1at = smp.tile([128, KL, L], MMDT, name="w1at", tag="w1at")
                    w2at = smp.tile([128, KL, L], MMDT, name="w2at", tag="w2at")
                    transpose256(w1a, w1at, ps_tr)
                    transpose256(w2a, w2at, ps_tr)
                    if stage < 4:
                        continue
                    # ---- att1T = sent2^T @ w1a^T ; att2T = sent1^T @ w2a^T ----
                    for m in range(KE):
                        pa = ps_e.tile([128, L], dt.float32, name="pa", tag="pe1")
                        mm_group(pa,
                                 lambda k, m=m: x2n_sb[:, gi * KL + k,
                                                       m * 128:(m + 1) * 128],
                                 lambda k: w1at[:, k], KL)
                        nc.vector.tensor_copy(att1[:, m, gl], pa)
                        pb = ps_e.tile([128, L], dt.float32, name="pb", tag="pe1")
                        mm_group(pb,
                                 lambda k, m=m: x1n_sb[:, gi * KL + k,
                                                       m * 128:(m + 1) * 128],
                                 lambda k: w2at[:, k], KL)
                        nc.vector.tensor_copy(att2[:, m, gl], pb)
                if "att1" in taps and bg == 0:
                    nc.sync.dma_start(out=taps["att1"][:],
                                      in_=att1.bitcast(dt.float32)[:, :, 0:L])

                if stage < 5:
                    continue
                # ---- compare MLP g (concat via psum accumulation over 2*KE) ----
                mlp2(pools,
                     lambda k: att1[:, k] if k < KE else x1t_sb[:, k - KE],
                     2 * KE, gw1_sb, gb1_sb, gw2_sb, gb2_sb, None, GL,
                     accum_cols=lambda gi, m, bs=bs: s_allT[:, m, bs[gi]:bs[gi] + 1])
                mlp2(pools,
                     lambda k: att2[:, k] if k < KE else x2t_sb[:, k - KE],
                     2 * KE, gw1_sb, gb1_sb, gw2_sb, gb2_sb, None, GL,
                     accum_cols=lambda gi, m, bs=bs: s_allT[:, m + KH, bs[gi]:bs[gi] + 1])

        # ---------------- tail: aggregate MLP + final linear ----------------
        if stage < 6:
            with tc.tile_pool(name="stub", bufs=1) as stub:
                z = stub.tile([4, nb], dt.float32)
                nc.vector.memset(z, 0.0)
                nc.sync.dma_start(out=out_d[:], in_=z)
            stage_tail = False
        else:
            stage_tail = True
        if stage_tail:
            with tc.tile_pool(name="tailw", bufs=1) as tw, \
                 tc.tile_pool(name="ps_t", bufs=4, space="PSUM") as ps_t:
                hw1_sb = tw.tile([128, 2 * KH, H], MMDT)
                hw2_sb = tw.tile([128, KH, H], MMDT)
                finw_sb = tw.tile([128, KH, 4], MMDT)
                nc.sync.dma_start(out=hw1_sb,
                                  in_=hw1t[:].rearrange("(k p) m -> p k m", p=128))
                nc.sync.dma_start(out=hw2_sb,
                                  in_=hw2t[:].rearrange("(k p) m -> p k m", p=128))
                nc.sync.dma_start(out=finw_sb,
                                  in_=finwt[:].rearrange("(k p) m -> p k m", p=128))
                hb1_sb = tw.tile([128, KH], dt.float32)
                hb2_sb = tw.tile([128, KH], dt.float32)
                finb_sb = tw.tile([4, 1], dt.float32)
                nc.sync.dma_start(out=hb1_sb, in_=hb1[:])
                nc.sync.dma_start(out=hb2_sb, in_=hb2[:])
                nc.sync.dma_start(out=finb_sb, in_=finb[:])

                s_r = tw.tile([128, 2 * KH, nb], MMDT)
                nc.vector.tensor_copy(s_r, s_allT)
                h1a = tw.tile([128, KH, nb], MMDT)
                for m in range(KH):
                    pst = ps_t.tile([128, nb], dt.float32, name="pst", tag="pst")
                    mm_group(pst, lambda k, m=m: hw1_sb[:, k, m * 128:(m + 1) * 128],
                             lambda k: s_r[:, k], 2 * KH)
                    nc.scalar.activation(out=h1a[:, m], in_=pst, func=AF.Relu,
                                         bias=hb1_sb[:, m:m + 1], scale=1.0)
                h2a = tw.tile([128, KH, nb], MMDT)
                for m in range(KH):
                    pst = ps_t.tile([128, nb], dt.float32, name="pst", tag="pst")
                    mm_group(pst, lambda k, m=m: hw2_sb[:, k, m * 128:(m + 1) * 128],
                             lambda k: h1a[:, k], KH)
                    nc.scalar.activation(out=h2a[:, m], in_=pst, func=AF.Relu,
                                         bias=hb2_sb[:, m:m + 1], scale=1.0)
                pfin = ps_t.tile([4, nb], dt.float32, name="pfin", tag="pfin")
                mm_group(pfin, lambda k: finw_sb[:, k], lambda k: h2a[:, k], KH)
                out_sb = tw.tile([4, nb], dt.float32)
                nc.scalar.activation(out=out_sb, in_=pfin, func=AF.Identity,
                                     bias=finb_sb, scale=1.0)
                nc.sync.dma_start(out=out_d[:], in_=out_sb)

    nc.finalize()
    return nc


def host_inputs(inputs, nb=NB, cores=NCORES):
    """Build per-core in_maps from the full problem inputs."""
    r = round_f32r
    s1 = np.ascontiguousarray(inputs["sent1"], dtype=np.float32)[:cores * nb]
    s2 = np.ascontiguousarray(inputs["sent2"], dtype=np.float32)[:cores * nb]
    x1n = r(s1).reshape(cores, nb, L, E)
    x2n = r(s2).reshape(cores, nb, L, E)
    x1tt = r(np.ascontiguousarray(np.swapaxes(s1, 1, 2))).reshape(cores, nb, E, L)
    x2tt = r(np.ascontiguousarray(np.swapaxes(s2, 1, 2))).reshape(cores, nb, E, L)

    def wt(w):  # [out, in] -> transposed [in, out], rounded
        return r(np.ascontiguousarray(np.asarray(w, np.float32).T))

    def bias_tiles(bvec):
        return np.ascontiguousarray(np.asarray(bvec, np.float32).reshape(KH, 128).T)

    finw = np.zeros((4, H), np.float32)
    finw[:OUT] = np.asarray(inputs["fin_w"], np.float32)
    finb = np.zeros((4, 1), np.float32)
    finb[:OUT, 0] = np.asarray(inputs["fin_b"], np.float32)

    shared = {
        "fw1t": wt(inputs["f_w1"]), "fw2t": wt(inputs["f_w2"]),
        "gw1t": wt(inputs["g_w1"]), "gw2t": wt(inputs["g_w2"]),
        "hw1t": wt(inputs["h_w1"]), "hw2t": wt(inputs["h_w2"]),
        "finwt": wt(finw),
        "fb1": bias_tiles(inputs["f_b1"]), "fb2": bias_tiles(inputs["f_b2"]),
        "gb1": bias_tiles(inputs["g_b1"]), "gb2": bias_tiles(inputs["g_b2"]),
        "hb1": bias_tiles(inputs["h_b1"]), "hb2": bias_tiles(inputs["h_b2"]),
        "finb": finb,
        "ident_in": np.eye(128, dtype=np.float32),
    }
    return [
        {"x1t": x1tt[c], "x2t": x2tt[c], "x1n": x1n[c], "x2n": x2n[c], **shared}
        for c in range(cores)
    ]


def assemble_output(results):
    outs = [res["out"].T[:, :OUT] for res in results]   # [nb, 3] each
    return np.ascontiguousarray(np.concatenate(outs, axis=0), dtype=np.float32)


# ----------------------------------------------------------------------------
# Public entry point: kernel(**inputs) -> [128, 3] float32
# ----------------------------------------------------------------------------
from concourse.bass_utils import run_bass_kernel_spmd

_NC_CACHE = {}


def _get_nc():
    key = (NB, 1)
    if key not in _NC_CACHE:
        _NC_CACHE[key] = build_nc(nb=NB, g=1)
    return _NC_CACHE[key]


def kernel(**inputs):
    nc = _get_nc()
    in_maps = host_inputs(inputs, nb=NB, cores=NCORES)
    res = run_bass_kernel_spmd(nc, in_maps, list(range(NCORES)))
    return assemble_output(res.results)



# revision 4
# speedup vs baseline: 29.9661x; 29.9661x over previous
"""Builder for the DecomposableAttention Trainium2 kernel.

Layouts (per core, NB batches), packed into 3 input blobs to minimize
per-dispatch argument overhead through the axon/PJRT tunnel:

  xblob [nb, 4, 512, 256] f32r : slot0 x1t [E,L], slot1 x2t [E,L],
                                 slot2 x1n [L,E] (flat-viewed as [512,256]),
                                 slot3 x2n likewise.
  wblob [6788, 1024]      f32r : fw1t|fw2t|gw1t|gw2t|hw1t|hw2t stacked on
                                 rows, then finw [4,1024] and ident.
  bblob [128, 49]         f32  : six bias tile-sets [128,8] + finb col.

All matmuls run in float32r (fp32 with low 12 mantissa bits zero).  PE
accumulates fp32 into PSUM.  Host pre-rounds every DRAM input; every
on-chip producer feeding a matmul writes a float32r-typed tile.

g = batches per MLP weight pass: the f/g MLP moving operands are g*L wide,
amortizing weight loads over g batches.  e1 / softmax / att stay per-batch.
"""

import sys

for p in ("/opt/trn_rl_repo", "/root/.axon_site/_ro/trn_rl_repo"):
    if p not in sys.path:
        sys.path.insert(0, p)

import numpy as np
import concourse.bass as bass
import concourse.mybir as mybir
from concourse import bacc
from concourse.tile import TileContext

dt = mybir.dt
AF = mybir.ActivationFunctionType
AX = mybir.AxisListType

B, L, E, H, OUT = 128, 256, 512, 1024, 3
NCORES = 8
NB = B // NCORES          # batches per core
KE = E // 128             # 4  k-tiles over E
KH = H // 128             # 8  k-tiles over H
KL = L // 128             # 2  k-tiles over L
MMDT = dt.float32r

# wblob row offsets
_OFF_FW1, _OFF_FW2 = 0, 512
_OFF_GW1, _OFF_GW2 = 1536, 2560
_OFF_HW1, _OFF_HW2 = 3584, 5632
_OFF_FIN, _OFF_ID = 6656, 6660
_WROWS = 6788
# bblob col offsets: fb1 fb2 gb1 gb2 hb1 hb2 | finb
_BCOLS = 49


def round_f32r(x: np.ndarray) -> np.ndarray:
    """Round fp32 array to the FP32R grid (11-bit mantissa, RNE)."""
    xi = np.ascontiguousarray(x, dtype=np.float32).view(np.uint32).astype(np.uint64)
    lsb = (xi >> 12) & 1
    r = (xi + 0x7FF + lsb) & 0xFFFFF000
    return r.astype(np.uint32).view(np.float32)


def build_nc(nb=NB, g=1):
    assert nb % g == 0
    nc = bacc.Bacc("TRN2", target_bir_lowering=False)
    GL = g * L

    xblob = nc.declare_dram_parameter("xblob", [nb, 4, 512, 256], MMDT,
                                      isOutput=False)
    wblob = nc.declare_dram_parameter("wblob", [_WROWS, 1024], MMDT,
                                      isOutput=False)
    bblob = nc.declare_dram_parameter("bblob", [128, _BCOLS], dt.float32,
                                      isOutput=False)
    out_d = nc.declare_dram_parameter("out", [4, nb], dt.float32, isOutput=True)

    def wrows(off, rows):
        return wblob[off:off + rows, :].rearrange("(k p) m -> p k m", p=128)

    with TileContext(nc) as tc, \
         tc.tile_pool(name="wpool", bufs=1) as wpool:
        ident = wpool.tile([128, 128], MMDT)
        nc.sync.dma_start(out=ident, in_=wblob[_OFF_ID:_OFF_ID + 128, 0:128])
        s_allT = wpool.tile([128, 2 * KH, nb], dt.float32)  # aggregate input

        def mm_group(psum, lhs_fn, rhs_fn, nk):
            for k in range(nk):
                nc.tensor.matmul(psum, lhs_fn(k), rhs_fn(k),
                                 start=(k == 0), stop=(k == nk - 1))

        def mlp2(pools, rhs1_fn, nk1, w1_sb, b1_sb, w2_sb, b2_sb, out_sb,
                 width, accum_cols=None):
            """Two-layer ReLU MLP on transposed activations ([128, k, width]).
            accum_cols(gi, m) -> [128, 1] AP: layer-2 relu row-sums per batch."""
            ps, hidp, scr = pools
            hid = hidp.tile([128, KH, width], MMDT, name="mlp_hid", tag="mlp_hid")
            for m in range(KH):
                psum = ps.tile([128, width], dt.float32, name="mlp_ps", tag="mlp_ps")
                mm_group(psum, lambda k, m=m: w1_sb[:, k, m * 128:(m + 1) * 128],
                         rhs1_fn, nk1)
                nc.scalar.activation(out=hid[:, m], in_=psum, func=AF.Relu,
                                     bias=b1_sb[:, m:m + 1], scale=1.0)
            for m in range(KH):
                psum = ps.tile([128, width], dt.float32, name="mlp_ps", tag="mlp_ps")
                mm_group(psum, lambda k, m=m: w2_sb[:, k, m * 128:(m + 1) * 128],
                         lambda k: hid[:, k], KH)
                if accum_cols is None:
                    nc.scalar.activation(out=out_sb[:, m], in_=psum, func=AF.Relu,
                                         bias=b2_sb[:, m:m + 1], scale=1.0)
                else:
                    for gi in range(width // L):
                        o = scr.tile([128, L], dt.float32, name="g_scr",
                                     tag="g_scr", bufs=3)
                        nc.scalar.activation(
                            out=o, in_=psum[:, gi * L:(gi + 1) * L], func=AF.Relu,
                            bias=b2_sb[:, m:m + 1], scale=1.0,
                            accum_out=accum_cols(gi, m))

        def transpose256(in_sb2, out_sb2, ps_tr):
            """[128, KL, 256] -> full 256x256 transpose via 4 PE transposes.
            Returns the psum tiles; copies to out_sb2 when given."""
            outs = []
            for a in range(KL):
                ptr = ps_tr.tile([128, L], MMDT, name="tr_ps", tag="tr_ps")
                for bq in range(KL):
                    nc.tensor.transpose(ptr[:, bq * 128:(bq + 1) * 128],
                                        in_sb2[:, bq, a * 128:(a + 1) * 128], ident)
                outs.append(ptr)
                if out_sb2 is not None:
                    nc.vector.tensor_copy(out_sb2[:, a], ptr)
            return outs

        with tc.tile_pool(name="fgw", bufs=1) as fgw, \
             tc.tile_pool(name="xp", bufs=2) as xp, \
             tc.tile_pool(name="xnp", bufs=2) as xnp, \
             tc.tile_pool(name="fp", bufs=1) as fp, \
             tc.tile_pool(name="hidp", bufs=2) as hidp, \
             tc.tile_pool(name="smp", bufs=1) as smp, \
             tc.tile_pool(name="attp", bufs=1) as attp, \
             tc.tile_pool(name="scrp", bufs=3) as scrp, \
             tc.tile_pool(name="ps", bufs=4, space="PSUM") as ps, \
             tc.tile_pool(name="ps_e", bufs=2, space="PSUM") as ps_e, \
             tc.tile_pool(name="ps_tr", bufs=2, space="PSUM") as ps_tr:
            pools = (ps, hidp, scrp)
            fw1_sb = fgw.tile([128, KE, H], MMDT)
            fw2_sb = fgw.tile([128, KH, H], MMDT)
            gw1_sb = fgw.tile([128, KH, H], MMDT)
            gw2_sb = fgw.tile([128, KH, H], MMDT)
            nc.sync.dma_start(out=fw1_sb, in_=wrows(_OFF_FW1, 512))
            nc.sync.dma_start(out=fw2_sb, in_=wrows(_OFF_FW2, 1024))
            nc.sync.dma_start(out=gw1_sb, in_=wrows(_OFF_GW1, 1024))
            nc.sync.dma_start(out=gw2_sb, in_=wrows(_OFF_GW2, 1024))
            bias_sb = fgw.tile([128, 4 * KH], dt.float32)
            nc.sync.dma_start(out=bias_sb, in_=bblob[:, 0:4 * KH])
            fb1_sb = bias_sb[:, 0 * KH:1 * KH]
            fb2_sb = bias_sb[:, 1 * KH:2 * KH]
            gb1_sb = bias_sb[:, 2 * KH:3 * KH]
            gb2_sb = bias_sb[:, 3 * KH:4 * KH]
            for bg in range(nb // g):
                bs = [bg * g + i for i in range(g)]
                # ---- load inputs ----
                x1t_sb = xp.tile([128, KE, GL], MMDT, name="x1t_sb", tag="x1t")
                x2t_sb = xp.tile([128, KE, GL], MMDT, name="x2t_sb", tag="x2t")
                x1n_sb = xnp.tile([128, KL * g, E], MMDT, name="x1n_sb", tag="x1n")
                x2n_sb = xnp.tile([128, KL * g, E], MMDT, name="x2n_sb", tag="x2n")
                for gi, b in enumerate(bs):
                    nc.sync.dma_start(
                        out=x1t_sb[:, :, gi * L:(gi + 1) * L],
                        in_=xblob[b, 0].rearrange("(k p) l -> p k l", p=128))
                    nc.sync.dma_start(
                        out=x2t_sb[:, :, gi * L:(gi + 1) * L],
                        in_=xblob[b, 1].rearrange("(k p) l -> p k l", p=128))
                    nc.sync.dma_start(
                        out=x1n_sb[:, gi * KL:(gi + 1) * KL, :],
                        in_=xblob[b, 2].rearrange("(k p a) y -> p k (a y)",
                                                  k=KL, p=128, a=2))
                    nc.sync.dma_start(
                        out=x2n_sb[:, gi * KL:(gi + 1) * KL, :],
                        in_=xblob[b, 3].rearrange("(k p a) y -> p k (a y)",
                                                  k=KL, p=128, a=2))

                # ---- attend MLP f (both sentences, g batches wide) ----
                f1t = fp.tile([128, KH, GL], MMDT, name="f1t", tag="f1t")
                f2t = fp.tile([128, KH, GL], MMDT, name="f2t", tag="f2t")
                mlp2(pools, lambda k: x1t_sb[:, k], KE, fw1_sb, fb1_sb,
                     fw2_sb, fb2_sb, f1t, GL)
                mlp2(pools, lambda k: x2t_sb[:, k], KE, fw1_sb, fb1_sb,
                     fw2_sb, fb2_sb, f2t, GL)

                # per-batch attention; att tiles span the group (GL wide)
                att1 = attp.tile([128, KE, GL], MMDT, name="att1", tag="att1")
                att2 = attp.tile([128, KE, GL], MMDT, name="att2", tag="att2")
                for gi, b in enumerate(bs):
                    gl = slice(gi * L, (gi + 1) * L)
                    # ---- e1 = f1 @ f2^T ----
                    e1sb = smp.tile([128, KL, L], MMDT, name="e1sb", tag="e1sb")
                    w1a = smp.tile([128, KL, L], MMDT, name="w1a", tag="w1a")
                    for im in range(KL):
                        pe1 = ps_e.tile([128, L], dt.float32, name="pe1", tag="pe1")
                        mm_group(pe1,
                                 lambda k, im=im: f1t[:, k, gi * L + im * 128:
                                                      gi * L + (im + 1) * 128],
                                 lambda k: f2t[:, k, gl], KH)
                        # row softmax (over j = free dim)
                        nmax = scrp.tile([128, 1], dt.float32, name="nmax", tag="sm1")
                        nc.vector.reduce_max(out=nmax, in_=pe1, axis=AX.X, negate=True)
                        ex = scrp.tile([128, L], dt.float32, name="ex", tag="smE")
                        rs = scrp.tile([128, 1], dt.float32, name="rs", tag="sm2")
                        nc.scalar.activation(out=ex, in_=pe1, func=AF.Exp,
                                             bias=nmax, scale=1.0, accum_out=rs)
                        rr = scrp.tile([128, 1], dt.float32, name="rr", tag="sm3")
                        nc.vector.reciprocal(out=rr, in_=rs)
                        nc.vector.tensor_scalar_mul(w1a[:, im], ex, rr)
                        nc.vector.tensor_copy(e1sb[:, im], pe1)
                    # ---- e1T -> col softmax -> w2a [j, i] ----
                    w2a = smp.tile([128, KL, L], MMDT, name="w2a", tag="w2a")
                    e1t_ps = transpose256(e1sb, None, ps_tr)
                    for jm in range(KL):
                        pet = e1t_ps[jm]
                        nmax2 = scrp.tile([128, 1], dt.float32, name="nmax2", tag="sm1")
                        nc.vector.reduce_max(out=nmax2, in_=pet, axis=AX.X, negate=True)
                        ex2 = scrp.tile([128, L], dt.float32, name="ex2", tag="smE")
                        cs = scrp.tile([128, 1], dt.float32, name="cs", tag="sm2")
                        nc.scalar.activation(out=ex2, in_=pet, func=AF.Exp,
                                             bias=nmax2, scale=1.0, accum_out=cs)
                        rc = scrp.tile([128, 1], dt.float32, name="rc", tag="sm3")
                        nc.vector.reciprocal(out=rc, in_=cs)
                        nc.vector.tensor_scalar_mul(w2a[:, jm], ex2, rc)
                    # ---- transpose weights for att matmuls ----
                    w1at = smp.tile([128, KL, L], MMDT, name="w1at", tag="w1at")
                    w2at = smp.tile([128, KL, L], MMDT, name="w2at", tag="w2at")
                    transpose256(w1a, w1at, ps_tr)
                    transpose256(w2a, w2at, ps_tr)
                    # ---- att1T = sent2^T @ w1a^T ; att2T = sent1^T @ w2a^T ----
                    for m in range(KE):
                        pa = ps_e.tile([128, L], dt.float32, name="pa", tag="pe1")
                        mm_group(pa,
                                 lambda k, m=m: x2n_sb[:, gi * KL + k,
                                                       m * 128:(m + 1) * 128],
                                 lambda k: w1at[:, k], KL)
                        nc.vector.tensor_copy(att1[:, m, gl], pa)
                        pb = ps_e.tile([128, L], dt.float32, name="pb", tag="pe1")
                        mm_group(pb,
                                 lambda k, m=m: x1n_sb[:, gi * KL + k,
                                                       m * 128:(m + 1) * 128],
                                 lambda k: w2at[:, k], KL)
                        nc.vector.tensor_copy(att2[:, m, gl], pb)

                # ---- compare MLP g (concat via psum accumulation over 2*KE) ----
                mlp2(pools,
                     lambda k: att1[:, k] if k < KE else x1t_sb[:, k - KE],
                     2 * KE, gw1_sb, gb1_sb, gw2_sb, gb2_sb, None, GL,
                     accum_cols=lambda gi, m, bs=bs: s_allT[:, m, bs[gi]:bs[gi] + 1])
                mlp2(pools,
                     lambda k: att2[:, k] if k < KE else x2t_sb[:, k - KE],
                     2 * KE, gw1_sb, gb1_sb, gw2_sb, gb2_sb, None, GL,
                     accum_cols=lambda gi, m, bs=bs: s_allT[:, m + KH, bs[gi]:bs[gi] + 1])

        # ---------------- tail: aggregate MLP + final linear ----------------
        with tc.tile_pool(name="tailw", bufs=1) as tw, \
             tc.tile_pool(name="ps_t", bufs=4, space="PSUM") as ps_t:
            hw1_sb = tw.tile([128, 2 * KH, H], MMDT)
            hw2_sb = tw.tile([128, KH, H], MMDT)
            finw_sb = tw.tile([128, KH, 4], MMDT)
            nc.sync.dma_start(out=hw1_sb, in_=wrows(_OFF_HW1, 2048))
            nc.sync.dma_start(out=hw2_sb, in_=wrows(_OFF_HW2, 1024))
            nc.sync.dma_start(
                out=finw_sb,
                in_=wblob[_OFF_FIN:_OFF_FIN + 4, :].rearrange(
                    "a (c p m) -> p (a c) m", c=2, p=128, m=4))
            hbias_sb = tw.tile([128, 2 * KH], dt.float32)
            nc.sync.dma_start(out=hbias_sb, in_=bblob[:, 4 * KH:6 * KH])
            hb1_sb = hbias_sb[:, 0:KH]
            hb2_sb = hbias_sb[:, KH:2 * KH]
            finb_sb = tw.tile([4, 1], dt.float32)
            nc.sync.dma_start(out=finb_sb, in_=bblob[0:4, 48:49])

            s_r = tw.tile([128, 2 * KH, nb], MMDT)
            nc.vector.tensor_copy(s_r, s_allT)
            h1a = tw.tile([128, KH, nb], MMDT)
            for m in range(KH):
                pst = ps_t.tile([128, nb], dt.float32, name="pst", tag="pst")
                mm_group(pst, lambda k, m=m: hw1_sb[:, k, m * 128:(m + 1) * 128],
                         lambda k: s_r[:, k], 2 * KH)
                nc.scalar.activation(out=h1a[:, m], in_=pst, func=AF.Relu,
                                     bias=hb1_sb[:, m:m + 1], scale=1.0)
            h2a = tw.tile([128, KH, nb], MMDT)
            for m in range(KH):
                pst = ps_t.tile([128, nb], dt.float32, name="pst", tag="pst")
                mm_group(pst, lambda k, m=m: hw2_sb[:, k, m * 128:(m + 1) * 128],
                         lambda k: h1a[:, k], KH)
                nc.scalar.activation(out=h2a[:, m], in_=pst, func=AF.Relu,
                                     bias=hb2_sb[:, m:m + 1], scale=1.0)
            pfin = ps_t.tile([4, nb], dt.float32, name="pfin", tag="pfin")
            mm_group(pfin, lambda k: finw_sb[:, k], lambda k: h2a[:, k], KH)
            out_sb = tw.tile([4, nb], dt.float32)
            nc.scalar.activation(out=out_sb, in_=pfin, func=AF.Identity,
                                 bias=finb_sb, scale=1.0)
            nc.sync.dma_start(out=out_d[:], in_=out_sb)

    nc.finalize()
    return nc


def host_inputs(inputs, nb=NB, cores=NCORES):
    """Build per-core in_maps (blob-packed) from the full problem inputs."""
    r = round_f32r
    s1 = np.ascontiguousarray(inputs["sent1"], dtype=np.float32)[:cores * nb]
    s2 = np.ascontiguousarray(inputs["sent2"], dtype=np.float32)[:cores * nb]
    xblob = np.empty((cores * nb, 4, 512, 256), np.float32)
    xblob[:, 0] = r(np.swapaxes(s1, 1, 2))                  # x1t [E, L]
    xblob[:, 1] = r(np.swapaxes(s2, 1, 2))                  # x2t [E, L]
    xblob[:, 2] = r(s1).reshape(cores * nb, 512, 256)       # x1n flat view
    xblob[:, 3] = r(s2).reshape(cores * nb, 512, 256)       # x2n flat view
    xblob = xblob.reshape(cores, nb, 4, 512, 256)

    def wt(w):  # [out, in] -> transposed [in, out], rounded
        return r(np.ascontiguousarray(np.asarray(w, np.float32).T))

    wblob = np.zeros((_WROWS, 1024), np.float32)
    wblob[_OFF_FW1:_OFF_FW1 + 512] = wt(inputs["f_w1"])
    wblob[_OFF_FW2:_OFF_FW2 + 1024] = wt(inputs["f_w2"])
    wblob[_OFF_GW1:_OFF_GW1 + 1024] = wt(inputs["g_w1"])
    wblob[_OFF_GW2:_OFF_GW2 + 1024] = wt(inputs["g_w2"])
    wblob[_OFF_HW1:_OFF_HW1 + 2048] = wt(inputs["h_w1"])
    wblob[_OFF_HW2:_OFF_HW2 + 1024] = wt(inputs["h_w2"])
    finw = np.zeros((4, H), np.float32)
    finw[:OUT] = np.asarray(inputs["fin_w"], np.float32)
    wblob[_OFF_FIN:_OFF_FIN + 4] = wt(finw).reshape(4, 1024)  # [H,4] flat
    wblob[_OFF_ID:_OFF_ID + 128, 0:128] = np.eye(128, dtype=np.float32)

    def bias_tiles(bvec):
        return np.asarray(bvec, np.float32).reshape(KH, 128).T

    bblob = np.zeros((128, _BCOLS), np.float32)
    bblob[:, 0:8] = bias_tiles(inputs["f_b1"])
    bblob[:, 8:16] = bias_tiles(inputs["f_b2"])
    bblob[:, 16:24] = bias_tiles(inputs["g_b1"])
    bblob[:, 24:32] = bias_tiles(inputs["g_b2"])
    bblob[:, 32:40] = bias_tiles(inputs["h_b1"])
    bblob[:, 40:48] = bias_tiles(inputs["h_b2"])
    bblob[0:OUT, 48] = np.asarray(inputs["fin_b"], np.float32)

    return [
        {"xblob": xblob[c], "wblob": wblob, "bblob": bblob}
        for c in range(cores)
    ]


def assemble_output(results):
    outs = [res["out"].T[:, :OUT] for res in results]   # [nb, 3] each
    return np.ascontiguousarray(np.concatenate(outs, axis=0), dtype=np.float32)


# ----------------------------------------------------------------------------
# Public entry point: kernel(**inputs) -> [128, 3] float32
# ----------------------------------------------------------------------------
from concourse.bass_utils import run_bass_kernel_spmd

_NC_CACHE = {}


def _get_nc():
    key = (NB, 1)
    if key not in _NC_CACHE:
        _NC_CACHE[key] = build_nc(nb=NB, g=1)
    return _NC_CACHE[key]


def kernel(**inputs):
    nc = _get_nc()
    in_maps = host_inputs(inputs, nb=NB, cores=NCORES)
    res = run_bass_kernel_spmd(nc, in_maps, list(range(NCORES)))
    return assemble_output(res.results)


# revision 6
# speedup vs baseline: 48.0880x; 1.6048x over previous
"""Builder for the DecomposableAttention Trainium2 kernel.

Layouts (per core, NB batches), packed into 3 input blobs to minimize
per-dispatch argument overhead through the axon/PJRT tunnel:

  xblob [nb, 4, 512, 256] f32r : slot0 x1t [E,L], slot1 x2t [E,L],
                                 slot2 x1n [L,E] (flat-viewed as [512,256]),
                                 slot3 x2n likewise.
  wblob [6788, 1024]      f32r : fw1t|fw2t|gw1t|gw2t|hw1t|hw2t stacked on
                                 rows, then finw [4,1024] and ident.
  bblob [128, 49]         f32  : six bias tile-sets [128,8] + finb col.

All matmuls run in float32r (fp32 with low 12 mantissa bits zero).  PE
accumulates fp32 into PSUM.  Host pre-rounds every DRAM input; every
on-chip producer feeding a matmul writes a float32r-typed tile.

g = batches per MLP weight pass: the f/g MLP moving operands are g*L wide,
amortizing weight loads over g batches.  e1 / softmax / att stay per-batch.
"""

import sys

for p in ("/opt/trn_rl_repo", "/root/.axon_site/_ro/trn_rl_repo"):
    if p not in sys.path:
        sys.path.insert(0, p)

import numpy as np
import concourse.bass as bass
import concourse.mybir as mybir
from concourse import bacc
from concourse.tile import TileContext

dt = mybir.dt
AF = mybir.ActivationFunctionType
AX = mybir.AxisListType

B, L, E, H, OUT = 128, 256, 512, 1024, 3
NCORES = 8
NB = B // NCORES          # batches per core
G = 1                     # batches per MLP weight pass
KE = E // 128             # 4  k-tiles over E
KH = H // 128             # 8  k-tiles over H
KL = L // 128             # 2  k-tiles over L
MMDT = dt.float32r

# wblob row offsets
_OFF_FW1, _OFF_FW2 = 0, 512
_OFF_GW1, _OFF_GW2 = 1536, 2560
_OFF_HW1, _OFF_HW2 = 3584, 5632
_OFF_FIN, _OFF_ID = 6656, 6660
_WROWS = 6788
# bblob col offsets: fb1 fb2 gb1 gb2 hb1 hb2 | finb
_BCOLS = 49


def round_f32r(x: np.ndarray) -> np.ndarray:
    """Round fp32 array to the FP32R grid (11-bit mantissa, RNE)."""
    xi = np.ascontiguousarray(x, dtype=np.float32).view(np.uint32).astype(np.uint64)
    lsb = (xi >> 12) & 1
    r = (xi + 0x7FF + lsb) & 0xFFFFF000
    return r.astype(np.uint32).view(np.float32)


def build_nc(nb=NB, g=1, repeat=1):
    """repeat>1 wraps the whole per-core program in a hardware loop that
    re-runs the identical computation; used by the timing harness to
    amortize dispatch overhead on-device.  Output is idempotent."""
    assert nb % g == 0
    nc = bacc.Bacc("TRN2", target_bir_lowering=False)
    GL = g * L

    xblob = nc.declare_dram_parameter("xblob", [nb, 4, 512, 256], MMDT,
                                      isOutput=False)
    wblob = nc.declare_dram_parameter("wblob", [_WROWS, 1024], MMDT,
                                      isOutput=False)
    bblob = nc.declare_dram_parameter("bblob", [128, _BCOLS], dt.float32,
                                      isOutput=False)
    out_d = nc.declare_dram_parameter("out", [4, nb], dt.float32, isOutput=True)

    def wrows(off, rows):
        return wblob[off:off + rows, :].rearrange("(k p) m -> p k m", p=128)

    from contextlib import ExitStack
    with TileContext(nc) as tc, \
         tc.tile_pool(name="wpool", bufs=1) as wpool, \
         ExitStack() as rep_ctx:
        if repeat > 1:
            rep_ctx.enter_context(tc.For_i(0, repeat, 1, name="rep"))
        ident = wpool.tile([128, 128], MMDT)
        nc.sync.dma_start(out=ident, in_=wblob[_OFF_ID:_OFF_ID + 128, 0:128])
        s_allT = wpool.tile([128, 2 * KH, nb], dt.float32)  # aggregate input

        def mm_group(psum, lhs_fn, rhs_fn, nk):
            for k in range(nk):
                nc.tensor.matmul(psum, lhs_fn(k), rhs_fn(k),
                                 start=(k == 0), stop=(k == nk - 1))

        def mlp2(pools, rhs1_fn, nk1, w1_sb, b1_sb, w2_sb, b2_sb, out_sb,
                 width, accum_cols=None):
            """Two-layer ReLU MLP on transposed activations ([128, k, width]).
            accum_cols(gi, m) -> [128, 1] AP: layer-2 relu row-sums per batch."""
            ps, hidp, scr = pools
            hid = hidp.tile([128, KH, width], MMDT, name="mlp_hid", tag="mlp_hid")
            for m in range(KH):
                psum = ps.tile([128, width], dt.float32, name="mlp_ps", tag="mlp_ps")
                mm_group(psum, lambda k, m=m: w1_sb[:, k, m * 128:(m + 1) * 128],
                         rhs1_fn, nk1)
                nc.scalar.activation(out=hid[:, m], in_=psum, func=AF.Relu,
                                     bias=b1_sb[:, m:m + 1], scale=1.0)
            for m in range(KH):
                psum = ps.tile([128, width], dt.float32, name="mlp_ps", tag="mlp_ps")
                mm_group(psum, lambda k, m=m: w2_sb[:, k, m * 128:(m + 1) * 128],
                         lambda k: hid[:, k], KH)
                if accum_cols is None:
                    nc.scalar.activation(out=out_sb[:, m], in_=psum, func=AF.Relu,
                                         bias=b2_sb[:, m:m + 1], scale=1.0)
                else:
                    for gi in range(width // L):
                        o = scr.tile([128, L], dt.float32, name="g_scr",
                                     tag="g_scr", bufs=3)
                        nc.scalar.activation(
                            out=o, in_=psum[:, gi * L:(gi + 1) * L], func=AF.Relu,
                            bias=b2_sb[:, m:m + 1], scale=1.0,
                            accum_out=accum_cols(gi, m))

        def transpose256(in_sb2, out_sb2, ps_tr):
            """[128, KL, 256] -> full 256x256 transpose via 4 PE transposes.
            Returns the psum tiles; copies to out_sb2 when given."""
            outs = []
            for a in range(KL):
                ptr = ps_tr.tile([128, L], MMDT, name="tr_ps", tag="tr_ps")
                for bq in range(KL):
                    nc.tensor.transpose(ptr[:, bq * 128:(bq + 1) * 128],
                                        in_sb2[:, bq, a * 128:(a + 1) * 128], ident)
                outs.append(ptr)
                if out_sb2 is not None:
                    nc.vector.tensor_copy(out_sb2[:, a], ptr)
            return outs

        with tc.tile_pool(name="fgw", bufs=1) as fgw, \
             tc.tile_pool(name="xp", bufs=2) as xp, \
             tc.tile_pool(name="xnp", bufs=2) as xnp, \
             tc.tile_pool(name="fp", bufs=1) as fp, \
             tc.tile_pool(name="hidp", bufs=2) as hidp, \
             tc.tile_pool(name="smp", bufs=1) as smp, \
             tc.tile_pool(name="attp", bufs=1) as attp, \
             tc.tile_pool(name="scrp", bufs=3) as scrp, \
             tc.tile_pool(name="ps", bufs=4, space="PSUM") as ps, \
             tc.tile_pool(name="ps_e", bufs=2, space="PSUM") as ps_e, \
             tc.tile_pool(name="ps_tr", bufs=2, space="PSUM") as ps_tr:
            pools = (ps, hidp, scrp)
            fw1_sb = fgw.tile([128, KE, H], MMDT)
            fw2_sb = fgw.tile([128, KH, H], MMDT)
            gw1_sb = fgw.tile([128, KH, H], MMDT)
            gw2_sb = fgw.tile([128, KH, H], MMDT)
            nc.sync.dma_start(out=fw1_sb, in_=wrows(_OFF_FW1, 512))
            nc.sync.dma_start(out=fw2_sb, in_=wrows(_OFF_FW2, 1024))
            nc.sync.dma_start(out=gw1_sb, in_=wrows(_OFF_GW1, 1024))
            nc.sync.dma_start(out=gw2_sb, in_=wrows(_OFF_GW2, 1024))
            bias_sb = fgw.tile([128, 4 * KH], dt.float32)
            nc.sync.dma_start(out=bias_sb, in_=bblob[:, 0:4 * KH])
            fb1_sb = bias_sb[:, 0 * KH:1 * KH]
            fb2_sb = bias_sb[:, 1 * KH:2 * KH]
            gb1_sb = bias_sb[:, 2 * KH:3 * KH]
            gb2_sb = bias_sb[:, 3 * KH:4 * KH]
            for bg in range(nb // g):
                bs = [bg * g + i for i in range(g)]
                # ---- load inputs ----
                x1t_sb = xp.tile([128, KE, GL], MMDT, name="x1t_sb", tag="x1t")
                x2t_sb = xp.tile([128, KE, GL], MMDT, name="x2t_sb", tag="x2t")
                x1n_sb = xnp.tile([128, KL * g, E], MMDT, name="x1n_sb", tag="x1n")
                x2n_sb = xnp.tile([128, KL * g, E], MMDT, name="x2n_sb", tag="x2n")
                for gi, b in enumerate(bs):
                    nc.sync.dma_start(
                        out=x1t_sb[:, :, gi * L:(gi + 1) * L],
                        in_=xblob[b, 0].rearrange("(k p) l -> p k l", p=128))
                    nc.sync.dma_start(
                        out=x2t_sb[:, :, gi * L:(gi + 1) * L],
                        in_=xblob[b, 1].rearrange("(k p) l -> p k l", p=128))
                    nc.sync.dma_start(
                        out=x1n_sb[:, gi * KL:(gi + 1) * KL, :],
                        in_=xblob[b, 2].rearrange("(k p a) y -> p k (a y)",
                                                  k=KL, p=128, a=2))
                    nc.sync.dma_start(
                        out=x2n_sb[:, gi * KL:(gi + 1) * KL, :],
                        in_=xblob[b, 3].rearrange("(k p a) y -> p k (a y)",
                                                  k=KL, p=128, a=2))

                # ---- attend MLP f (both sentences, g batches wide) ----
                f1t = fp.tile([128, KH, GL], MMDT, name="f1t", tag="f1t")
                f2t = fp.tile([128, KH, GL], MMDT, name="f2t", tag="f2t")
                mlp2(pools, lambda k: x1t_sb[:, k], KE, fw1_sb, fb1_sb,
                     fw2_sb, fb2_sb, f1t, GL)
                mlp2(pools, lambda k: x2t_sb[:, k], KE, fw1_sb, fb1_sb,
                     fw2_sb, fb2_sb, f2t, GL)

                # per-batch attention; att tiles span the group (GL wide)
                att1 = attp.tile([128, KE, GL], MMDT, name="att1", tag="att1")
                att2 = attp.tile([128, KE, GL], MMDT, name="att2", tag="att2")
                for gi, b in enumerate(bs):
                    gl = slice(gi * L, (gi + 1) * L)
                    # ---- e1 = f1 @ f2^T ----
                    e1sb = smp.tile([128, KL, L], MMDT, name="e1sb", tag="e1sb")
                    w1a = smp.tile([128, KL, L], MMDT, name="w1a", tag="w1a")
                    for im in range(KL):
                        pe1 = ps_e.tile([128, L], dt.float32, name="pe1", tag="pe1")
                        mm_group(pe1,
                                 lambda k, im=im: f1t[:, k, gi * L + im * 128:
                                                      gi * L + (im + 1) * 128],
                                 lambda k: f2t[:, k, gl], KH)
                        # row softmax (over j = free dim)
                        nmax = scrp.tile([128, 1], dt.float32, name="nmax", tag="sm1")
                        nc.vector.reduce_max(out=nmax, in_=pe1, axis=AX.X, negate=True)
                        ex = scrp.tile([128, L], dt.float32, name="ex", tag="smE")
                        rs = scrp.tile([128, 1], dt.float32, name="rs", tag="sm2")
                        nc.scalar.activation(out=ex, in_=pe1, func=AF.Exp,
                                             bias=nmax, scale=1.0, accum_out=rs)
                        rr = scrp.tile([128, 1], dt.float32, name="rr", tag="sm3")
                        nc.vector.reciprocal(out=rr, in_=rs)
                        nc.vector.tensor_scalar_mul(w1a[:, im], ex, rr)
                        nc.vector.tensor_copy(e1sb[:, im], pe1)
                    # ---- e1T -> col softmax -> w2a [j, i] ----
                    w2a = smp.tile([128, KL, L], MMDT, name="w2a", tag="w2a")
                    e1t_ps = transpose256(e1sb, None, ps_tr)
                    for jm in range(KL):
                        pet = e1t_ps[jm]
                        nmax2 = scrp.tile([128, 1], dt.float32, name="nmax2", tag="sm1")
                        nc.vector.reduce_max(out=nmax2, in_=pet, axis=AX.X, negate=True)
                        ex2 = scrp.tile([128, L], dt.float32, name="ex2", tag="smE")
                        cs = scrp.tile([128, 1], dt.float32, name="cs", tag="sm2")
                        nc.scalar.activation(out=ex2, in_=pet, func=AF.Exp,
                                             bias=nmax2, scale=1.0, accum_out=cs)
                        rc = scrp.tile([128, 1], dt.float32, name="rc", tag="sm3")
                        nc.vector.reciprocal(out=rc, in_=cs)
                        nc.vector.tensor_scalar_mul(w2a[:, jm], ex2, rc)
                    # ---- transpose weights for att matmuls ----
                    w1at = smp.tile([128, KL, L], MMDT, name="w1at", tag="w1at")
                    w2at = smp.tile([128, KL, L], MMDT, name="w2at", tag="w2at")
                    transpose256(w1a, w1at, ps_tr)
                    transpose256(w2a, w2at, ps_tr)
                    # ---- att1T = sent2^T @ w1a^T ; att2T = sent1^T @ w2a^T ----
                    for m in range(KE):
                        pa = ps_e.tile([128, L], dt.float32, name="pa", tag="pe1")
                        mm_group(pa,
                                 lambda k, m=m: x2n_sb[:, gi * KL + k,
                                                       m * 128:(m + 1) * 128],
                                 lambda k: w1at[:, k], KL)
                        nc.vector.tensor_copy(att1[:, m, gl], pa)
                        pb = ps_e.tile([128, L], dt.float32, name="pb", tag="pe1")
                        mm_group(pb,
                                 lambda k, m=m: x1n_sb[:, gi * KL + k,
                                                       m * 128:(m + 1) * 128],
                                 lambda k: w2at[:, k], KL)
                        nc.vector.tensor_copy(att2[:, m, gl], pb)

                # ---- compare MLP g (concat via psum accumulation over 2*KE) ----
                mlp2(pools,
                     lambda k: att1[:, k] if k < KE else x1t_sb[:, k - KE],
                     2 * KE, gw1_sb, gb1_sb, gw2_sb, gb2_sb, None, GL,
                     accum_cols=lambda gi, m, bs=bs: s_allT[:, m, bs[gi]:bs[gi] + 1])
                mlp2(pools,
                     lambda k: att2[:, k] if k < KE else x2t_sb[:, k - KE],
                     2 * KE, gw1_sb, gb1_sb, gw2_sb, gb2_sb, None, GL,
                     accum_cols=lambda gi, m, bs=bs: s_allT[:, m + KH, bs[gi]:bs[gi] + 1])

        # ---------------- tail: aggregate MLP + final linear ----------------
        with tc.tile_pool(name="tailw", bufs=1) as tw, \
             tc.tile_pool(name="ps_t", bufs=4, space="PSUM") as ps_t:
            hw1_sb = tw.tile([128, 2 * KH, H], MMDT)
            hw2_sb = tw.tile([128, KH, H], MMDT)
            finw_sb = tw.tile([128, KH, 4], MMDT)
            nc.sync.dma_start(out=hw1_sb, in_=wrows(_OFF_HW1, 2048))
            nc.sync.dma_start(out=hw2_sb, in_=wrows(_OFF_HW2, 1024))
            nc.sync.dma_start(
                out=finw_sb,
                in_=wblob[_OFF_FIN:_OFF_FIN + 4, :].rearrange(
                    "a (c p m) -> p (a c) m", c=2, p=128, m=4))
            hbias_sb = tw.tile([128, 2 * KH], dt.float32)
            nc.sync.dma_start(out=hbias_sb, in_=bblob[:, 4 * KH:6 * KH])
            hb1_sb = hbias_sb[:, 0:KH]
            hb2_sb = hbias_sb[:, KH:2 * KH]
            finb_sb = tw.tile([4, 1], dt.float32)
            nc.sync.dma_start(out=finb_sb, in_=bblob[0:4, 48:49])

            s_r = tw.tile([128, 2 * KH, nb], MMDT)
            nc.vector.tensor_copy(s_r, s_allT)
            h1a = tw.tile([128, KH, nb], MMDT)
            for m in range(KH):
                pst = ps_t.tile([128, nb], dt.float32, name="pst", tag="pst")
                mm_group(pst, lambda k, m=m: hw1_sb[:, k, m * 128:(m + 1) * 128],
                         lambda k: s_r[:, k], 2 * KH)
                nc.scalar.activation(out=h1a[:, m], in_=pst, func=AF.Relu,
                                     bias=hb1_sb[:, m:m + 1], scale=1.0)
            h2a = tw.tile([128, KH, nb], MMDT)
            for m in range(KH):
                pst = ps_t.tile([128, nb], dt.float32, name="pst", tag="pst")
                mm_group(pst, lambda k, m=m: hw2_sb[:, k, m * 128:(m + 1) * 128],
                         lambda k: h1a[:, k], KH)
                nc.scalar.activation(out=h2a[:, m], in_=pst, func=AF.Relu,
                                     bias=hb2_sb[:, m:m + 1], scale=1.0)
            pfin = ps_t.tile([4, nb], dt.float32, name="pfin", tag="pfin")
            mm_group(pfin, lambda k: finw_sb[:, k], lambda k: h2a[:, k], KH)
            out_sb = tw.tile([4, nb], dt.float32)
            nc.scalar.activation(out=out_sb, in_=pfin, func=AF.Identity,
                                 bias=finb_sb, scale=1.0)
            nc.sync.dma_start(out=out_d[:], in_=out_sb)

    nc.finalize()
    return nc


def host_inputs(inputs, nb=NB, cores=NCORES):
    """Build per-core in_maps (blob-packed) from the full problem inputs."""
    r = round_f32r
    s1 = np.ascontiguousarray(inputs["sent1"], dtype=np.float32)[:cores * nb]
    s2 = np.ascontiguousarray(inputs["sent2"], dtype=np.float32)[:cores * nb]
    xblob = np.empty((cores * nb, 4, 512, 256), np.float32)
    xblob[:, 0] = r(np.swapaxes(s1, 1, 2))                  # x1t [E, L]
    xblob[:, 1] = r(np.swapaxes(s2, 1, 2))                  # x2t [E, L]
    xblob[:, 2] = r(s1).reshape(cores * nb, 512, 256)       # x1n flat view
    xblob[:, 3] = r(s2).reshape(cores * nb, 512, 256)       # x2n flat view
    xblob = xblob.reshape(cores, nb, 4, 512, 256)

    def wt(w):  # [out, in] -> transposed [in, out], rounded
        return r(np.ascontiguousarray(np.asarray(w, np.float32).T))

    wblob = np.zeros((_WROWS, 1024), np.float32)
    wblob[_OFF_FW1:_OFF_FW1 + 512] = wt(inputs["f_w1"])
    wblob[_OFF_FW2:_OFF_FW2 + 1024] = wt(inputs["f_w2"])
    wblob[_OFF_GW1:_OFF_GW1 + 1024] = wt(inputs["g_w1"])
    wblob[_OFF_GW2:_OFF_GW2 + 1024] = wt(inputs["g_w2"])
    wblob[_OFF_HW1:_OFF_HW1 + 2048] = wt(inputs["h_w1"])
    wblob[_OFF_HW2:_OFF_HW2 + 1024] = wt(inputs["h_w2"])
    finw = np.zeros((4, H), np.float32)
    finw[:OUT] = np.asarray(inputs["fin_w"], np.float32)
    wblob[_OFF_FIN:_OFF_FIN + 4] = wt(finw).reshape(4, 1024)  # [H,4] flat
    wblob[_OFF_ID:_OFF_ID + 128, 0:128] = np.eye(128, dtype=np.float32)

    def bias_tiles(bvec):
        return np.asarray(bvec, np.float32).reshape(KH, 128).T

    bblob = np.zeros((128, _BCOLS), np.float32)
    bblob[:, 0:8] = bias_tiles(inputs["f_b1"])
    bblob[:, 8:16] = bias_tiles(inputs["f_b2"])
    bblob[:, 16:24] = bias_tiles(inputs["g_b1"])
    bblob[:, 24:32] = bias_tiles(inputs["g_b2"])
    bblob[:, 32:40] = bias_tiles(inputs["h_b1"])
    bblob[:, 40:48] = bias_tiles(inputs["h_b2"])
    bblob[0:OUT, 48] = np.asarray(inputs["fin_b"], np.float32)

    return [
        {"xblob": xblob[c], "wblob": wblob, "bblob": bblob}
        for c in range(cores)
    ]


def assemble_output(results):
    outs = [res["out"].T[:, :OUT] for res in results]   # [nb, 3] each
    return np.ascontiguousarray(np.concatenate(outs, axis=0), dtype=np.float32)


# ----------------------------------------------------------------------------
# Public entry point: kernel(**inputs) -> [128, 3] float32
# ----------------------------------------------------------------------------
from concourse.bass_utils import run_bass_kernel_spmd

_NC_CACHE = {}


def _get_nc():
    key = (NB, 1)
    if key not in _NC_CACHE:
        _NC_CACHE[key] = build_nc(nb=NB, g=1)
    return _NC_CACHE[key]


def kernel(**inputs):
    nc = _get_nc()
    in_maps = host_inputs(inputs, nb=NB, cores=NCORES)
    res = run_bass_kernel_spmd(nc, in_maps, list(range(NCORES)))
    return assemble_output(res.results)


# revision 10
# speedup vs baseline: 49.0581x; 1.0202x over previous
"""Builder for the DecomposableAttention Trainium2 kernel.

Layouts (per core, NB batches), packed into 3 input blobs to minimize
per-dispatch argument overhead through the axon/PJRT tunnel:

  xblob [nb, 4, 512, 256] bf16 : slot0 x1t [E,L], slot1 x2t [E,L],
                                 slot2 x1n [L,E] (flat-viewed as [512,256]),
                                 slot3 x2n likewise.
  wblob [6788, 1024]      bf16 : fw1t|fw2t|gw1t|gw2t|hw1t|hw2t stacked on
                                 rows, then finw [4,1024] and ident.
  bblob [128, 49]         f32  : six bias tile-sets [128,8] + finb col.

All matmuls run in bfloat16 (1 cycle/row on the PE, half the SBUF and
DMA footprint of fp32).  PE accumulates fp32 into PSUM; softmax math is
done on the fp32 PSUM values; biases stay fp32 through the activation
engine.

g = batches per MLP weight pass: the f/g MLP moving operands are g*L wide,
amortizing weight loads over g batches.  e1 / softmax / att stay per-batch.
"""

import sys

for p in ("/opt/trn_rl_repo", "/root/.axon_site/_ro/trn_rl_repo"):
    if p not in sys.path:
        sys.path.insert(0, p)

import numpy as np
import concourse.bass as bass
import concourse.mybir as mybir
from concourse import bacc
from concourse.tile import TileContext

dt = mybir.dt
AF = mybir.ActivationFunctionType
AX = mybir.AxisListType

B, L, E, H, OUT = 128, 256, 512, 1024, 3
NCORES = 8
NB = B // NCORES          # batches per core
G = 2                     # batches per MLP weight pass
KE = E // 128             # 4  k-tiles over E
KH = H // 128             # 8  k-tiles over H
KL = L // 128             # 2  k-tiles over L
MMDT = dt.bfloat16

# wblob row offsets
_OFF_FW1, _OFF_FW2 = 0, 512
_OFF_GW1, _OFF_GW2 = 1536, 2560
_OFF_HW1, _OFF_HW2 = 3584, 5632
_OFF_FIN, _OFF_ID = 6656, 6660
_WROWS = 6788
# bblob col offsets: fb1 fb2 gb1 gb2 hb1 hb2 | finb
_BCOLS = 49


def round_f32r(x: np.ndarray) -> np.ndarray:
    """Round fp32 array to the FP32R grid (11-bit mantissa, RNE)."""
    xi = np.ascontiguousarray(x, dtype=np.float32).view(np.uint32).astype(np.uint64)
    lsb = (xi >> 12) & 1
    r = (xi + 0x7FF + lsb) & 0xFFFFF000
    return r.astype(np.uint32).view(np.float32)


def build_nc(nb=NB, g=1, repeat=1):
    """repeat>1 wraps the whole per-core program in a hardware loop that
    re-runs the identical computation; used by the timing harness to
    amortize dispatch overhead on-device.  Output is idempotent."""
    assert nb % g == 0
    nc = bacc.Bacc("TRN2", target_bir_lowering=False)
    GL = g * L

    xblob = nc.declare_dram_parameter("xblob", [nb, 4, 512, 256], MMDT,
                                      isOutput=False)
    wblob = nc.declare_dram_parameter("wblob", [_WROWS, 1024], MMDT,
                                      isOutput=False)
    bblob = nc.declare_dram_parameter("bblob", [128, _BCOLS], dt.float32,
                                      isOutput=False)
    out_d = nc.declare_dram_parameter("out", [4, nb], dt.float32, isOutput=True)

    def wrows(off, rows):
        return wblob[off:off + rows, :].rearrange("(k p) m -> p k m", p=128)

    from contextlib import ExitStack
    with TileContext(nc) as tc, \
         tc.tile_pool(name="wpool", bufs=1) as wpool, \
         ExitStack() as rep_ctx:
        if repeat > 1:
            rep_ctx.enter_context(tc.For_i(0, repeat, 1, name="rep"))
        ident = wpool.tile([128, 128], MMDT)
        nc.sync.dma_start(out=ident, in_=wblob[_OFF_ID:_OFF_ID + 128, 0:128])
        s_allT = wpool.tile([128, 2 * KH, nb], dt.float32)  # aggregate input

        def mm_group(psum, lhs_fn, rhs_fn, nk):
            for k in range(nk):
                nc.tensor.matmul(psum, lhs_fn(k), rhs_fn(k),
                                 start=(k == 0), stop=(k == nk - 1))

        def mlp2(pools, rhs1_fn, nk1, w1_sb, b1_sb, w2_sb, b2_sb, out_sb,
                 width, accum_cols=None):
            """Two-layer ReLU MLP on transposed activations ([128, k, width]).
            accum_cols(gi, m) -> [128, 1] AP: layer-2 relu row-sums per batch."""
            ps, hidp, scr = pools
            hid = hidp.tile([128, KH, width], MMDT, name="mlp_hid", tag="mlp_hid")
            for m in range(KH):
                psum = ps.tile([128, width], dt.float32, name="mlp_ps", tag="mlp_ps")
                mm_group(psum, lambda k, m=m: w1_sb[:, k, m * 128:(m + 1) * 128],
                         rhs1_fn, nk1)
                nc.scalar.activation(out=hid[:, m], in_=psum, func=AF.Relu,
                                     bias=b1_sb[:, m:m + 1], scale=1.0)
            for m in range(KH):
                psum = ps.tile([128, width], dt.float32, name="mlp_ps", tag="mlp_ps")
                mm_group(psum, lambda k, m=m: w2_sb[:, k, m * 128:(m + 1) * 128],
                         lambda k: hid[:, k], KH)
                if accum_cols is None:
                    nc.scalar.activation(out=out_sb[:, m], in_=psum, func=AF.Relu,
                                         bias=b2_sb[:, m:m + 1], scale=1.0)
                else:
                    for gi in range(width // L):
                        o = scr.tile([128, L], dt.float32, name="g_scr",
                                     tag="g_scr", bufs=3)
                        nc.scalar.activation(
                            out=o, in_=psum[:, gi * L:(gi + 1) * L], func=AF.Relu,
                            bias=b2_sb[:, m:m + 1], scale=1.0,
                            accum_out=accum_cols(gi, m))

        def transpose256(in_sb2, out_sb2, ps_tr):
            """[128, KL, 256] -> full 256x256 transpose via 4 PE transposes.
            Returns the psum tiles; copies to out_sb2 when given."""
            outs = []
            for a in range(KL):
                ptr = ps_tr.tile([128, L], MMDT, name="tr_ps", tag="tr_ps")
                for bq in range(KL):
                    nc.tensor.transpose(ptr[:, bq * 128:(bq + 1) * 128],
                                        in_sb2[:, bq, a * 128:(a + 1) * 128], ident)
                outs.append(ptr)
                if out_sb2 is not None:
                    nc.vector.tensor_copy(out_sb2[:, a], ptr)
            return outs

        with tc.tile_pool(name="fgw", bufs=1) as fgw, \
             tc.tile_pool(name="xp", bufs=2) as xp, \
             tc.tile_pool(name="xnp", bufs=2) as xnp, \
             tc.tile_pool(name="fp", bufs=1) as fp, \
             tc.tile_pool(name="hidp", bufs=2) as hidp, \
             tc.tile_pool(name="smp", bufs=1) as smp, \
             tc.tile_pool(name="attp", bufs=1) as attp, \
             tc.tile_pool(name="scrp", bufs=3) as scrp, \
             tc.tile_pool(name="ps", bufs=4, space="PSUM") as ps, \
             tc.tile_pool(name="ps_e", bufs=2, space="PSUM") as ps_e, \
             tc.tile_pool(name="ps_tr", bufs=2, space="PSUM") as ps_tr:
            pools = (ps, hidp, scrp)
            fw1_sb = fgw.tile([128, KE, H], MMDT)
            fw2_sb = fgw.tile([128, KH, H], MMDT)
            gw1_sb = fgw.tile([128, KH, H], MMDT)
            gw2_sb = fgw.tile([128, KH, H], MMDT)
            nc.sync.dma_start(out=fw1_sb, in_=wrows(_OFF_FW1, 512))
            nc.sync.dma_start(out=fw2_sb, in_=wrows(_OFF_FW2, 1024))
            nc.sync.dma_start(out=gw1_sb, in_=wrows(_OFF_GW1, 1024))
            nc.sync.dma_start(out=gw2_sb, in_=wrows(_OFF_GW2, 1024))
            bias_sb = fgw.tile([128, 4 * KH], dt.float32)
            nc.sync.dma_start(out=bias_sb, in_=bblob[:, 0:4 * KH])
            fb1_sb = bias_sb[:, 0 * KH:1 * KH]
            fb2_sb = bias_sb[:, 1 * KH:2 * KH]
            gb1_sb = bias_sb[:, 2 * KH:3 * KH]
            gb2_sb = bias_sb[:, 3 * KH:4 * KH]
            for bg in range(nb // g):
                bs = [bg * g + i for i in range(g)]
                # ---- load inputs ----
                x1t_sb = xp.tile([128, KE, GL], MMDT, name="x1t_sb", tag="x1t")
                x2t_sb = xp.tile([128, KE, GL], MMDT, name="x2t_sb", tag="x2t")
                x1n_sb = xnp.tile([128, KL * g, E], MMDT, name="x1n_sb", tag="x1n")
                x2n_sb = xnp.tile([128, KL * g, E], MMDT, name="x2n_sb", tag="x2n")
                for gi, b in enumerate(bs):
                    nc.sync.dma_start(
                        out=x1t_sb[:, :, gi * L:(gi + 1) * L],
                        in_=xblob[b, 0].rearrange("(k p) l -> p k l", p=128))
                    nc.sync.dma_start(
                        out=x2t_sb[:, :, gi * L:(gi + 1) * L],
                        in_=xblob[b, 1].rearrange("(k p) l -> p k l", p=128))
                    nc.sync.dma_start(
                        out=x1n_sb[:, gi * KL:(gi + 1) * KL, :],
                        in_=xblob[b, 2].rearrange("(k p a) y -> p k (a y)",
                                                  k=KL, p=128, a=2))
                    nc.sync.dma_start(
                        out=x2n_sb[:, gi * KL:(gi + 1) * KL, :],
                        in_=xblob[b, 3].rearrange("(k p a) y -> p k (a y)",
                                                  k=KL, p=128, a=2))

                # ---- attend MLP f (both sentences, g batches wide) ----
                f1t = fp.tile([128, KH, GL], MMDT, name="f1t", tag="f1t")
                f2t = fp.tile([128, KH, GL], MMDT, name="f2t", tag="f2t")
                mlp2(pools, lambda k: x1t_sb[:, k], KE, fw1_sb, fb1_sb,
                     fw2_sb, fb2_sb, f1t, GL)
                mlp2(pools, lambda k: x2t_sb[:, k], KE, fw1_sb, fb1_sb,
                     fw2_sb, fb2_sb, f2t, GL)

                # per-batch attention; att tiles span the group (GL wide)
                att1 = attp.tile([128, KE, GL], MMDT, name="att1", tag="att1")
                att2 = attp.tile([128, KE, GL], MMDT, name="att2", tag="att2")
                for gi, b in enumerate(bs):
                    gl = slice(gi * L, (gi + 1) * L)
                    # ---- e1 = f1 @ f2^T ----
                    e1sb = smp.tile([128, KL, L], MMDT, name="e1sb", tag="e1sb")
                    w1a = smp.tile([128, KL, L], MMDT, name="w1a", tag="w1a")
                    for im in range(KL):
                        pe1 = ps_e.tile([128, L], dt.float32, name="pe1", tag="pe1")
                        mm_group(pe1,
                                 lambda k, im=im: f1t[:, k, gi * L + im * 128:
                                                      gi * L + (im + 1) * 128],
                                 lambda k: f2t[:, k, gl], KH)
                        # row softmax (over j = free dim)
                        nmax = scrp.tile([128, 1], dt.float32, name="nmax", tag="sm1")
                        nc.vector.reduce_max(out=nmax, in_=pe1, axis=AX.X, negate=True)
                        ex = scrp.tile([128, L], dt.float32, name="ex", tag="smE")
                        rs = scrp.tile([128, 1], dt.float32, name="rs", tag="sm2")
                        nc.scalar.activation(out=ex, in_=pe1, func=AF.Exp,
                                             bias=nmax, scale=1.0, accum_out=rs)
                        rr = scrp.tile([128, 1], dt.float32, name="rr", tag="sm3")
                        nc.vector.reciprocal(out=rr, in_=rs)
                        nc.vector.tensor_scalar_mul(w1a[:, im], ex, rr)
                        nc.vector.tensor_copy(e1sb[:, im], pe1)
                    # ---- e1T -> col softmax -> w2a [j, i] ----
                    w2a = smp.tile([128, KL, L], MMDT, name="w2a", tag="w2a")
                    e1t_ps = transpose256(e1sb, None, ps_tr)
                    for jm in range(KL):
                        pet = e1t_ps[jm]
                        nmax2 = scrp.tile([128, 1], dt.float32, name="nmax2", tag="sm1")
                        nc.vector.reduce_max(out=nmax2, in_=pet, axis=AX.X, negate=True)
                        ex2 = scrp.tile([128, L], dt.float32, name="ex2", tag="smE")
                        cs = scrp.tile([128, 1], dt.float32, name="cs", tag="sm2")
                        nc.scalar.activation(out=ex2, in_=pet, func=AF.Exp,
                                             bias=nmax2, scale=1.0, accum_out=cs)
                        rc = scrp.tile([128, 1], dt.float32, name="rc", tag="sm3")
                        nc.vector.reciprocal(out=rc, in_=cs)
                        nc.vector.tensor_scalar_mul(w2a[:, jm], ex2, rc)
                    # ---- transpose weights for att matmuls ----
                    w1at = smp.tile([128, KL, L], MMDT, name="w1at", tag="w1at")
                    w2at = smp.tile([128, KL, L], MMDT, name="w2at", tag="w2at")
                    transpose256(w1a, w1at, ps_tr)
                    transpose256(w2a, w2at, ps_tr)
                    # ---- att1T = sent2^T @ w1a^T ; att2T = sent1^T @ w2a^T ----
                    for m in range(KE):
                        pa = ps_e.tile([128, L], dt.float32, name="pa", tag="pe1")
                        mm_group(pa,
                                 lambda k, m=m: x2n_sb[:, gi * KL + k,
                                                       m * 128:(m + 1) * 128],
                                 lambda k: w1at[:, k], KL)
                        nc.vector.tensor_copy(att1[:, m, gl], pa)
                        pb = ps_e.tile([128, L], dt.float32, name="pb", tag="pe1")
                        mm_group(pb,
                                 lambda k, m=m: x1n_sb[:, gi * KL + k,
                                                       m * 128:(m + 1) * 128],
                                 lambda k: w2at[:, k], KL)
                        nc.vector.tensor_copy(att2[:, m, gl], pb)

                # ---- compare MLP g (concat via psum accumulation over 2*KE) ----
                mlp2(pools,
                     lambda k: att1[:, k] if k < KE else x1t_sb[:, k - KE],
                     2 * KE, gw1_sb, gb1_sb, gw2_sb, gb2_sb, None, GL,
                     accum_cols=lambda gi, m, bs=bs: s_allT[:, m, bs[gi]:bs[gi] + 1])
                mlp2(pools,
                     lambda k: att2[:, k] if k < KE else x2t_sb[:, k - KE],
                     2 * KE, gw1_sb, gb1_sb, gw2_sb, gb2_sb, None, GL,
                     accum_cols=lambda gi, m, bs=bs: s_allT[:, m + KH, bs[gi]:bs[gi] + 1])

        # ---------------- tail: aggregate MLP + final linear ----------------
        with tc.tile_pool(name="tailw", bufs=1) as tw, \
             tc.tile_pool(name="ps_t", bufs=4, space="PSUM") as ps_t:
            hw1_sb = tw.tile([128, 2 * KH, H], MMDT)
            hw2_sb = tw.tile([128, KH, H], MMDT)
            finw_sb = tw.tile([128, KH, 4], MMDT)
            nc.sync.dma_start(out=hw1_sb, in_=wrows(_OFF_HW1, 2048))
            nc.sync.dma_start(out=hw2_sb, in_=wrows(_OFF_HW2, 1024))
            nc.sync.dma_start(
                out=finw_sb,
                in_=wblob[_OFF_FIN:_OFF_FIN + 4, :].rearrange(
                    "a (c p m) -> p (a c) m", c=2, p=128, m=4))
            hbias_sb = tw.tile([128, 2 * KH], dt.float32)
            nc.sync.dma_start(out=hbias_sb, in_=bblob[:, 4 * KH:6 * KH])
            hb1_sb = hbias_sb[:, 0:KH]
            hb2_sb = hbias_sb[:, KH:2 * KH]
            finb_sb = tw.tile([4, 1], dt.float32)
            nc.sync.dma_start(out=finb_sb, in_=bblob[0:4, 48:49])

            s_r = tw.tile([128, 2 * KH, nb], MMDT)
            nc.vector.tensor_copy(s_r, s_allT)
            h1a = tw.tile([128, KH, nb], MMDT)
            for m in range(KH):
                pst = ps_t.tile([128, nb], dt.float32, name="pst", tag="pst")
                mm_group(pst, lambda k, m=m: hw1_sb[:, k, m * 128:(m + 1) * 128],
                         lambda k: s_r[:, k], 2 * KH)
                nc.scalar.activation(out=h1a[:, m], in_=pst, func=AF.Relu,
                                     bias=hb1_sb[:, m:m + 1], scale=1.0)
            h2a = tw.tile([128, KH, nb], MMDT)
            for m in range(KH):
                pst = ps_t.tile([128, nb], dt.float32, name="pst", tag="pst")
                mm_group(pst, lambda k, m=m: hw2_sb[:, k, m * 128:(m + 1) * 128],
                         lambda k: h1a[:, k], KH)
                nc.scalar.activation(out=h2a[:, m], in_=pst, func=AF.Relu,
                                     bias=hb2_sb[:, m:m + 1], scale=1.0)
            pfin = ps_t.tile([4, nb], dt.float32, name="pfin", tag="pfin")
            mm_group(pfin, lambda k: finw_sb[:, k], lambda k: h2a[:, k], KH)
            out_sb = tw.tile([4, nb], dt.float32)
            nc.scalar.activation(out=out_sb, in_=pfin, func=AF.Identity,
                                 bias=finb_sb, scale=1.0)
            nc.sync.dma_start(out=out_d[:], in_=out_sb)

    nc.finalize()
    return nc


def host_inputs(inputs, nb=NB, cores=NCORES):
    """Build per-core in_maps (blob-packed) from the full problem inputs."""
    import ml_dtypes
    BF16 = np.dtype(ml_dtypes.bfloat16)
    s1 = np.ascontiguousarray(inputs["sent1"], dtype=np.float32)[:cores * nb]
    s2 = np.ascontiguousarray(inputs["sent2"], dtype=np.float32)[:cores * nb]
    xblob = np.empty((cores * nb, 4, 512, 256), BF16)
    xblob[:, 0] = np.swapaxes(s1, 1, 2).astype(BF16)        # x1t [E, L]
    xblob[:, 1] = np.swapaxes(s2, 1, 2).astype(BF16)        # x2t [E, L]
    xblob[:, 2] = s1.reshape(cores * nb, 512, 256).astype(BF16)  # x1n flat
    xblob[:, 3] = s2.reshape(cores * nb, 512, 256).astype(BF16)  # x2n flat
    xblob = xblob.reshape(cores, nb, 4, 512, 256)

    def wt(w):  # [out, in] -> transposed [in, out]
        return np.ascontiguousarray(np.asarray(w, np.float32).T)

    wblob = np.zeros((_WROWS, 1024), BF16)
    wblob[_OFF_FW1:_OFF_FW1 + 512] = wt(inputs["f_w1"])
    wblob[_OFF_FW2:_OFF_FW2 + 1024] = wt(inputs["f_w2"])
    wblob[_OFF_GW1:_OFF_GW1 + 1024] = wt(inputs["g_w1"])
    wblob[_OFF_GW2:_OFF_GW2 + 1024] = wt(inputs["g_w2"])
    wblob[_OFF_HW1:_OFF_HW1 + 2048] = wt(inputs["h_w1"])
    wblob[_OFF_HW2:_OFF_HW2 + 1024] = wt(inputs["h_w2"])
    finw = np.zeros((4, H), np.float32)
    finw[:OUT] = np.asarray(inputs["fin_w"], np.float32)
    wblob[_OFF_FIN:_OFF_FIN + 4] = wt(finw).reshape(4, 1024)  # [H,4] flat
    wblob[_OFF_ID:_OFF_ID + 128, 0:128] = np.eye(128, dtype=np.float32)

    def bias_tiles(bvec):
        return np.asarray(bvec, np.float32).reshape(KH, 128).T

    bblob = np.zeros((128, _BCOLS), np.float32)
    bblob[:, 0:8] = bias_tiles(inputs["f_b1"])
    bblob[:, 8:16] = bias_tiles(inputs["f_b2"])
    bblob[:, 16:24] = bias_tiles(inputs["g_b1"])
    bblob[:, 24:32] = bias_tiles(inputs["g_b2"])
    bblob[:, 32:40] = bias_tiles(inputs["h_b1"])
    bblob[:, 40:48] = bias_tiles(inputs["h_b2"])
    bblob[0:OUT, 48] = np.asarray(inputs["fin_b"], np.float32)

    return [
        {"xblob": xblob[c], "wblob": wblob, "bblob": bblob}
        for c in range(cores)
    ]


def assemble_output(results):
    outs = [res["out"].T[:, :OUT] for res in results]   # [nb, 3] each
    return np.ascontiguousarray(np.concatenate(outs, axis=0), dtype=np.float32)


# ----------------------------------------------------------------------------
# Public entry point: kernel(**inputs) -> [128, 3] float32
# ----------------------------------------------------------------------------
from concourse.bass_utils import run_bass_kernel_spmd

_NC_CACHE = {}


def _get_nc():
    key = (NB, G)
    if key not in _NC_CACHE:
        _NC_CACHE[key] = build_nc(nb=NB, g=G)
    return _NC_CACHE[key]


def kernel(**inputs):
    nc = _get_nc()
    in_maps = host_inputs(inputs, nb=NB, cores=NCORES)
    res = run_bass_kernel_spmd(nc, in_maps, list(range(NCORES)))
    return assemble_output(res.results)
